# revision 61
# baseline (speedup 1.0000x reference)
"""Single-head attention (B=4, S=2048, E=1024, H=64) on 8 trn2 NeuronCores.

Sharding: core c -> batch b = c//2, query-half h = c%2.  Each core is
fully independent (no collectives -- a 0.5 MB pairwise AllGather costs
~2.5 ms on this axon fabric, dwarfing the whole kernel): it loads x[b]
(both halves, bf16), computes q for its own 1024 queries and k/v for
all 2048 keys, and writes its [1024, 64] output slice.

Primary layout "pipe" (bf16, collective-free, pipelined):
  - x arrives as 8 token-chunks of [128p, 8et, 256tok] (partition p,
    slot et hold embedding row 8p+et); projections chase the chunk
    DMAs (weights go first on the same sync ring so they can never
    queue behind the bulk x transfers).
  - Per chunk: kv psum [128,256] accumulates over 8 e-tiles
    (lhsT=[Wk|Wv]-tile, rhs=x-chunk); q likewise for own chunks.  DVE
    evictions add per-partition biases ([bk|0]; bv deferred to phase C).
  - v tiles: PE-transpose of kv rows 64:128 per k-tile, DVE-evicted
    with an appended ones column so the PV matmul also accumulates
    softmax row-sums.
  - Attention runs as two 512-query streams (j=0 then j=1), each one
    PV accumulation chain over the 16 k-tiles:
      S^T[k,q] = matmul(lhsT=k-tile[64,128], rhs=q^T[64,512]) -> psum
      P^T = exp(S^T/8), one ACT op per k-tile pair ([128,2,512];
      scores are small: no max subtraction needed)
      out'[65,512] += matmul(lhsT=[v|1][128,65], rhs=P^T[128,512])
    Stream j=0 closes early so its phase C + output DMA overlap
    stream j=1.
  - Phase C: PE-transpose out' -> [128q, 65], DVE reciprocal of the
    row-sum column, fused (out * 1/sum) + bv via scalar_tensor_tensor.
  PSUM budget (8 banks): proj/transpose pool 2 + scores 2x2 + PV 2.

Fallback layout "dup" (f32r) is the older collective-free design; "cc"
and "cc2" (sequence-parallel, pairwise collectives) are kept for
reference but are ~25x slower end-to-end on this fabric.
"""

import numpy as np
from contextlib import ExitStack

import concourse.bacc as bacc
import concourse.tile as tile
from concourse import mybir
from concourse.masks import make_identity

B, S, E, H = 4, 2048, 1024, 64
NCORES = 8
HALF = S // 2  # queries per core
ET = E // 128  # e-tiles
KT = S // 128  # k-tiles
F32 = mybir.dt.float32

SCALE = 0.125  # 1/sqrt(H)


def _emit_rep(nc, tc, DT, dram, consts, r, upto="full", layout="dup"):
    """One full iteration of the kernel body (DMA + all phases)."""
    xt, out = dram["xt"], dram["out"]
    wvq_sb, wk_sb, wvk_sb = consts["wvq"], consts["wk"], consts["wvk"]
    b0q_sb, b0k_sb, bvb_sb = consts["b0q"], consts["b0k"], consts["bvb"]
    ident, ident_r = consts["ident"], consts["ident_r"]

    with ExitStack() as ctx:
        persist = ctx.enter_context(tc.tile_pool(name=f"persist{r}", bufs=1))

        # --- x^T (e-permuted: partition p holds e-rows 8p..8p+7, one
        # 1024-col s-segment per e).  4 x 1MB DMA chunks per half. ---
        xt_own = persist.tile([128, ET * HALF], DT, tag="xt_own",
                              name=f"xt_own_{r}")
        xt_oth = persist.tile([128, ET * HALF], DT, tag="xt_oth",
                              name=f"xt_oth_{r}")
        CH = ET * HALF // 4  # 2048 elems per partition per chunk
        _engs = [nc.sync, nc.scalar, nc.gpsimd]
        for c in range(4):
            _engs[c % 3].dma_start(
                out=xt_own[:, c * CH:(c + 1) * CH], in_=xt[0, :, c, :])
        for c in range(4):
            _engs[(c + 1) % 3].dma_start(
                out=xt_oth[:, c * CH:(c + 1) * CH], in_=xt[1, :, c, :])

        def xo_slice(t, et, blk):
            o = et * HALF + blk * 512
            return t[:, o:o + 512]

        # --- projection outputs ---
        # vq_own rows 0:64 = v^T_own, rows 64:128 = q^T_own
        vq_own = persist.tile([128, HALF], DT, tag="vq_own", name=f"vq_own_{r}")
        # k_own rows 64:128 = k^T of own half
        k_own = persist.tile([128, HALF], DT, tag="k_own", name=f"k_own_{r}")
        # vk_oth rows 0:64 = v^T_oth, rows 64:128 = k^T_oth
        vk_oth = persist.tile([128, HALF], DT, tag="vk_oth", name=f"vk_oth_{r}")
        v_sb = [
            persist.tile([128, H + 1], DT, tag=f"v{kt}", name=f"v{kt}_{r}")
            for kt in range(KT)
        ]
        f_all = persist.tile(
            [128, 2 * HALF // 256, H], F32, tag="f_all", name=f"f_all_{r}"
        )

        if upto == "dma":
            # DMA only + one anchor matmul per half so DCE keeps the loads
            with tc.tile_pool(name=f"pad{r}", bufs=1, space="PSUM") as pad:
                psd = pad.tile([128, 512], F32, tag="d", name=f"psd_{r}")
                for et in range(ET):
                    nc.tensor.matmul(psd, wvq_sb[:, et], xo_slice(xt_own, et, 0),
                                     start=(et == 0), stop=(et == ET - 1))
                for et in range(ET):
                    nc.tensor.matmul(psd, wvq_sb[:, et], xo_slice(xt_oth, et, 0),
                                     start=(et == 0), stop=(et == ET - 1))
                nc.vector.tensor_scalar_add(vq_own[:, 0:512], psd, b0q_sb)
                nc.vector.memset(f_all, 0.0)
                nc.sync.dma_start(
                    out=out.rearrange("(t p) d -> p t d", p=128),
                    in_=f_all)
            return

        if layout == "cc2":
            # Like cc, but the pair exchange is AllReduce(add) + local
            # subtract (no rank-dependent slot selection), and attention over
            # the core's own 8 k-tiles starts before the exchange completes
            # (softmax accumulation is key-order invariant).
            kv_mine = persist.tile([128, HALF], DT, tag="kv_mine",
                                   name=f"kv_mine_{r}")
            kv_peer = persist.tile([128, HALF], DT, tag="kv_peer",
                                   name=f"kv_peer_{r}")
            q_own = persist.tile([H, HALF], DT, tag="q_own", name=f"q_own_{r}")
            with tc.tile_pool(name=f"pao{r}", bufs=1, space="PSUM") as pao:
                for blk in range(2):
                    ps_kv = pao.tile([128, 512], F32, tag=f"akv{blk}",
                                     name=f"akv{blk}_{r}")
                    ps_q = pao.tile([H, 512], F32, tag=f"aq{blk}",
                                    name=f"aq{blk}_{r}")
                    for et in range(ET):
                        st = dict(start=(et == 0), stop=(et == ET - 1))
                        xo = xo_slice(xt_own, et, blk)
                        nc.tensor.matmul(ps_kv, consts["wkv2"][:, et], xo, **st)
                        nc.tensor.matmul(ps_q, consts["wq"][:, et], xo, **st)
                    c0, c1 = blk * 512, (blk + 1) * 512
                    nc.vector.tensor_scalar_add(
                        kv_mine[:, c0:c1], ps_kv, consts["bkv"])
                    nc.vector.tensor_scalar_add(
                        q_own[:, c0:c1], ps_q, consts["bq64"])
            with tc.tile_pool(name=f"dr{r}", bufs=1, space="DRAM") as drp:
                kv_local = drp.tile([128, HALF], DT, tag="kvl",
                                    name=f"kv_local_{r}")
                kv_sum = drp.tile([128, HALF], DT, tag="kvs",
                                  name=f"kv_sum_{r}")
                nc.sync.dma_start(out=kv_local[:], in_=kv_mine)
                nc.gpsimd.collective_compute(
                    "AllReduce",
                    mybir.AluOpType.add,
                    replica_groups=[[0, 1], [2, 3], [4, 5], [6, 7]],
                    ins=[kv_local.opt()],
                    outs=[kv_sum.opt()],
                )
                kv_sum_sb = persist.tile([128, HALF], DT, tag="kv_sum_sb",
                                         name=f"kv_sum_sb_{r}")
                nc.scalar.dma_start(out=kv_sum_sb, in_=kv_sum[:])
                nc.vector.tensor_sub(kv_peer, kv_sum_sb, kv_mine)

            psS = ctx.enter_context(
                tc.tile_pool(name=f"psS{r}", bufs=2, space="PSUM"))
            psO = ctx.enter_context(
                tc.tile_pool(name=f"psO{r}", bufs=2, space="PSUM"))
            pt = ctx.enter_context(
                tc.tile_pool(name=f"pt{r}", bufs=2, space="PSUM"))
            ppool = ctx.enter_context(tc.tile_pool(name=f"pp{r}", bufs=3))
            opool = ctx.enter_context(tc.tile_pool(name=f"op{r}", bufs=4))

            def kvsrc(kt):
                t = kv_mine if kt < 8 else kv_peer
                kk = kt % 8
                return t, kk * 128, (kk + 1) * 128

            def v_transpose2(kt):
                t, a, b = kvsrc(kt)
                pst = pt.tile([128, H + 1], DT, tag="tr", name=f"pst{kt}_{r}")
                nc.tensor.transpose(
                    pst[:, 0:H], t[64:128, a:b], ident_r[64:128, 64:128])
                nc.vector.tensor_copy(v_sb[kt][:, 0:H], pst[:, 0:H])
                ones_col = v_sb[kt][:, H:H + 1]
                nc.vector.memset(
                    ones_col.bitcast(F32) if DT == mybir.dt.float32r
                    else ones_col, 1.0)

            for kt in range(8):
                v_transpose2(kt)

            for qb in range(HALF // 512):
                q_ap = q_own[:, qb * 512:(qb + 1) * 512]
                o_ps = psO.tile([H + 1, 512], F32, tag="o", name=f"o_ps{qb}_{r}")
                for g in range(KT // 2):
                    if qb == 0 and g == 4:
                        for kt in range(8, KT):
                            v_transpose2(kt)
                    s_ps = psS.tile([128, 2, 512], F32, tag="s",
                                    name=f"s_ps{qb}_{g}_{r}")
                    for j in range(2):
                        kt = g * 2 + j
                        t, a, b = kvsrc(kt)
                        nc.tensor.matmul(s_ps[:, j], t[0:64, a:b], q_ap,
                                         start=True, stop=True)
                    p_sb = ppool.tile([128, 2, 512], DT, tag="p",
                                      name=f"p_sb{qb}_{g}_{r}")
                    nc.scalar.activation(
                        p_sb, s_ps, mybir.ActivationFunctionType.Exp,
                        scale=SCALE)
                    for j in range(2):
                        kt = g * 2 + j
                        nc.tensor.matmul(
                            o_ps, v_sb[kt], p_sb[:, j],
                            start=(kt == 0), stop=(kt == KT - 1),
                        )
                for sub in range(4):
                    o_t = opool.tile([H + 1, 128], F32, tag="ot",
                                     name=f"ot{qb}{sub}_{r}")
                    nc.vector.tensor_copy(o_t, o_ps[:, sub * 128:(sub + 1) * 128])
                    ps2 = pt.tile([128, H + 1], F32, tag="tr",
                                  name=f"ps2_{qb}{sub}_{r}")
                    nc.tensor.transpose(ps2, o_t, ident[0:H + 1, 0:H + 1])
                    r_t = opool.tile([128, 1], F32, tag="rt",
                                     name=f"rt{qb}{sub}_{r}")
                    nc.vector.reciprocal(r_t, ps2[:, H:H + 1])
                    fa = f_all[:, qb * 4 + sub, :]
                    nc.vector.tensor_scalar_mul(fa, ps2[:, 0:H], r_t)
                    nc.vector.tensor_add(fa, fa, bvb_sb)

            nc.sync.dma_start(
                out=out.rearrange("(t p) d -> p t d", p=128), in_=f_all)
            return

        if layout == "cc":
            # Sequence-parallel projections: each core projects only its own
            # half (k^T rows 0:64 + bk, v^T rows 64:128, q^T separately),
            # then pairwise-AllGathers [k^T; v^T] so both cores hold the
            # full-sequence k/v.
            kv_mine = persist.tile([128, HALF], DT, tag="kv_mine",
                                   name=f"kv_mine_{r}")
            q_own = persist.tile([H, HALF], DT, tag="q_own", name=f"q_own_{r}")
            kv_all = persist.tile([128, 2, HALF], DT, tag="kv_all",
                                  name=f"kv_all_{r}")
            with tc.tile_pool(name=f"pao{r}", bufs=1, space="PSUM") as pao:
                for blk in range(2):
                    ps_kv = pao.tile([128, 512], F32, tag=f"akv{blk}",
                                     name=f"akv{blk}_{r}")
                    ps_q = pao.tile([H, 512], F32, tag=f"aq{blk}",
                                    name=f"aq{blk}_{r}")
                    for et in range(ET):
                        st = dict(start=(et == 0), stop=(et == ET - 1))
                        xo = xo_slice(xt_own, et, blk)
                        nc.tensor.matmul(ps_kv, consts["wkv2"][:, et], xo, **st)
                        nc.tensor.matmul(ps_q, consts["wq"][:, et], xo, **st)
                    c0, c1 = blk * 512, (blk + 1) * 512
                    nc.vector.tensor_scalar_add(
                        kv_mine[:, c0:c1], ps_kv, consts["bkv"])
                    nc.vector.tensor_scalar_add(
                        q_own[:, c0:c1], ps_q, consts["bq64"])
            with tc.tile_pool(name=f"dr{r}", bufs=1, space="DRAM") as drp:
                kv_local = drp.tile([128, HALF], DT, tag="kvl",
                                    name=f"kv_local_{r}")
                kv_pair = drp.tile([2, 128, HALF], DT, tag="kvp",
                                   name=f"kv_pair_{r}")
                nc.sync.dma_start(out=kv_local[:], in_=kv_mine)
                nc.gpsimd.collective_compute(
                    "AllGather",
                    mybir.AluOpType.bypass,
                    replica_groups=[[0, 1], [2, 3], [4, 5], [6, 7]],
                    ins=[kv_local.opt()],
                    outs=[kv_pair.opt()],
                )
                for s in range(2):
                    nc.sync.dma_start(out=kv_all[:, s, :], in_=kv_pair[s])

            psS = ctx.enter_context(
                tc.tile_pool(name=f"psS{r}", bufs=2, space="PSUM"))
            psO = ctx.enter_context(
                tc.tile_pool(name=f"psO{r}", bufs=1, space="PSUM"))
            pt = ctx.enter_context(
                tc.tile_pool(name=f"pt{r}", bufs=2, space="PSUM"))
            ppool = ctx.enter_context(tc.tile_pool(name=f"pp{r}", bufs=3))
            opool = ctx.enter_context(tc.tile_pool(name=f"op{r}", bufs=4))

            for kt in range(KT):
                slot, kk = kt // 8, kt % 8
                srcv = kv_all[64:128, slot, kk * 128:(kk + 1) * 128]
                pst = pt.tile([128, H + 1], DT, tag="tr", name=f"pst{kt}_{r}")
                nc.tensor.transpose(
                    pst[:, 0:H], srcv, ident_r[64:128, 64:128])
                nc.vector.tensor_copy(v_sb[kt][:, 0:H], pst[:, 0:H])
                ones_col = v_sb[kt][:, H:H + 1]
                nc.vector.memset(
                    ones_col.bitcast(F32) if DT == mybir.dt.float32r else ones_col,
                    1.0)

            for qb in range(HALF // 512):
                q_ap = q_own[:, qb * 512:(qb + 1) * 512]
                o_ps = psO.tile([H + 1, 512], F32, tag="o", name=f"o_ps{qb}_{r}")
                for g in range(KT // 2):
                    s_ps = psS.tile([128, 2, 512], F32, tag="s",
                                    name=f"s_ps{qb}_{g}_{r}")
                    for j in range(2):
                        kt = g * 2 + j
                        slot, kk = kt // 8, kt % 8
                        kl = kv_all[0:64, slot, kk * 128:(kk + 1) * 128]
                        nc.tensor.matmul(s_ps[:, j], kl, q_ap,
                                         start=True, stop=True)
                    p_sb = ppool.tile([128, 2, 512], DT, tag="p",
                                      name=f"p_sb{qb}_{g}_{r}")
                    nc.scalar.activation(
                        p_sb, s_ps, mybir.ActivationFunctionType.Exp,
                        scale=SCALE)
                    for j in range(2):
                        kt = g * 2 + j
                        nc.tensor.matmul(
                            o_ps, v_sb[kt], p_sb[:, j],
                            start=(kt == 0), stop=(kt == KT - 1),
                        )
                for sub in range(4):
                    o_t = opool.tile([H + 1, 128], F32, tag="ot",
                                     name=f"ot{qb}{sub}_{r}")
                    nc.vector.tensor_copy(o_t, o_ps[:, sub * 128:(sub + 1) * 128])
                    ps2 = pt.tile([128, H + 1], F32, tag="tr",
                                  name=f"ps2_{qb}{sub}_{r}")
                    nc.tensor.transpose(ps2, o_t, ident[0:H + 1, 0:H + 1])
                    r_t = opool.tile([128, 1], F32, tag="rt",
                                     name=f"rt{qb}{sub}_{r}")
                    nc.vector.reciprocal(r_t, ps2[:, H:H + 1])
                    fa = f_all[:, qb * 4 + sub, :]
                    nc.vector.tensor_scalar_mul(fa, ps2[:, 0:H], r_t)
                    nc.vector.tensor_add(fa, fa, bvb_sb)

            nc.sync.dma_start(
                out=out.rearrange("(t p) d -> p t d", p=128), in_=f_all)
            return

        # ---------- Phase A (own half): q^T, k^T_own, v^T_own ----------
        with tc.tile_pool(name=f"pao{r}", bufs=1, space="PSUM") as pao:
            ps_vq = [
                pao.tile([128, 512], F32, tag=f"avq{i}", name=f"avq{i}_{r}")
                for i in range(2)
            ]
            ps_k = [
                pao.tile([128, 512], F32, tag=f"ak{i}", name=f"ak{i}_{r}")
                for i in range(2)
            ]
            for et in range(ET):
                st = dict(start=(et == 0), stop=(et == ET - 1))
                for blk in range(2):
                    xo = xo_slice(xt_own, et, blk)
                    nc.tensor.matmul(ps_vq[blk], wvq_sb[:, et], xo, **st)
                    nc.tensor.matmul(ps_k[blk], wk_sb[:, et], xo, **st)
            for blk in range(2):
                c0, c1 = blk * 512, (blk + 1) * 512
                nc.vector.tensor_scalar_add(vq_own[:, c0:c1], ps_vq[blk], b0q_sb)
                nc.vector.tensor_scalar_add(
                    k_own[64:128, c0:c1], ps_k[blk][64:128, :], b0k_sb[64:128, :]
                )

        # Remaining PSUM budget (8 banks): psS 4 + psO 1 + pt 2 + pa2 1
        psS = ctx.enter_context(tc.tile_pool(name=f"psS{r}", bufs=2, space="PSUM"))
        psO = ctx.enter_context(tc.tile_pool(name=f"psO{r}", bufs=1, space="PSUM"))
        pt = ctx.enter_context(tc.tile_pool(name=f"pt{r}", bufs=2, space="PSUM"))
        pa2 = ctx.enter_context(tc.tile_pool(name=f"pa2{r}", bufs=1, space="PSUM"))
        ppool = ctx.enter_context(tc.tile_pool(name=f"pp{r}", bufs=3))
        opool = ctx.enter_context(tc.tile_pool(name=f"op{r}", bufs=4))

        def v_transpose(kt):
            src = (
                vq_own[0:64, kt * 128:(kt + 1) * 128]
                if kt < 8
                else vk_oth[0:64, (kt - 8) * 128:(kt - 7) * 128]
            )
            pst = pt.tile([128, H + 1], DT, tag="tr", name=f"pst{kt}_{r}")
            nc.tensor.transpose(pst[:, 0:H], src, ident_r[0:64, 0:64])
            nc.vector.tensor_copy(v_sb[kt][:, 0:H], pst[:, 0:H])
            ones_col = v_sb[kt][:, H:H + 1]
            nc.vector.memset(
                ones_col.bitcast(F32) if DT == mybir.dt.float32r else ones_col,
                1.0)

        # ---------- Phase A2 (own half v tiles) ----------
        for kt in range(8):
            v_transpose(kt)

        # ---------- Phase A (other half): k^T_oth, v^T_oth ----------
        for blk in range(2):
            ps_vk = pa2.tile([128, 512], F32, tag="avk", name=f"avk{blk}_{r}")
            for et in range(ET):
                xf = xo_slice(xt_oth, et, blk)
                nc.tensor.matmul(
                    ps_vk, wvk_sb[:, et], xf,
                    start=(et == 0), stop=(et == ET - 1),
                )
            nc.vector.tensor_scalar_add(
                vk_oth[:, blk * 512:(blk + 1) * 512], ps_vk, b0k_sb
            )
        for kt in range(8, KT):
            v_transpose(kt)

        if upto == "proj":
            nc.vector.memset(f_all, 0.0)
            nc.sync.dma_start(
                out=out.rearrange("(t p) d -> p t d", p=128), in_=f_all)
            return

        # ---------- Phase B + C: attention per query block ----------
        for qb in range(HALF // 512):
            q_ap = vq_own[64:128, qb * 512:(qb + 1) * 512]
            o_ps = psO.tile([H + 1, 512], F32, tag="o", name=f"o_ps{qb}_{r}")
            for g in range(KT // 2):
                s_ps = psS.tile([128, 2, 512], F32, tag="s", name=f"s_ps{qb}_{g}_{r}")
                for j in range(2):
                    kt = g * 2 + j
                    kl = (
                        k_own[64:128, kt * 128:(kt + 1) * 128]
                        if kt < 8
                        else vk_oth[64:128, (kt - 8) * 128:(kt - 7) * 128]
                    )
                    nc.tensor.matmul(s_ps[:, j], kl, q_ap, start=True, stop=True)
                p_sb = ppool.tile([128, 2, 512], DT, tag="p", name=f"p_sb{qb}_{g}_{r}")
                nc.scalar.activation(
                    p_sb, s_ps, mybir.ActivationFunctionType.Exp, scale=SCALE
                )
                for j in range(2):
                    kt = g * 2 + j
                    nc.tensor.matmul(
                        o_ps, v_sb[kt], p_sb[:, j],
                        start=(kt == 0), stop=(kt == KT - 1),
                    )
            if upto == "attn":
                nc.vector.tensor_copy(f_all[0:65, qb * 4, :], o_ps[:, 0:64])
                continue
            for sub in range(4):
                o_t = opool.tile([H + 1, 128], F32, tag="ot", name=f"ot{qb}{sub}_{r}")
                nc.vector.tensor_copy(o_t, o_ps[:, sub * 128:(sub + 1) * 128])
                ps2 = pt.tile([128, H + 1], F32, tag="tr", name=f"ps2_{qb}{sub}_{r}")
                nc.tensor.transpose(ps2, o_t, ident[0:H + 1, 0:H + 1])
                r_t = opool.tile([128, 1], F32, tag="rt", name=f"rt{qb}{sub}_{r}")
                nc.vector.reciprocal(r_t, ps2[:, H:H + 1])
                fa = f_all[:, qb * 4 + sub, :]
                nc.vector.tensor_scalar_mul(fa, ps2[:, 0:H], r_t)
                nc.vector.tensor_add(fa, fa, bvb_sb)

        nc.sync.dma_start(out=out.rearrange("(t p) d -> p t d", p=128), in_=f_all)


def _emit_pipe(nc, tc, DTX, DTA, dram, consts, r):
    """dup-style (collective-free) pipelined rep: token-chunked x DMA
    chased by projections; attention over own k-tiles interleaves with
    the other-half projections; single PV accumulation pair per core.

    DTX: dtype of x + projection weights (bf16 halves DMA but forces an
    InstLdweights per projection matmul).  DTA: dtype of kv/q/v/p SBUF
    storage (f32r keeps attention matmuls self-loading: no Ldweights).

    Per core: 1024 own queries, full 2048 keys.  PSUM budget (8 banks):
    pA 2 (proj kv/q + transposes) + ps 2x2 (scores pairs) + po 2 (PV).
    """
    xt4, out = dram["xt4"], dram["out"]
    wkv_sb, wq_sb = consts["wkv2"], consts["wq"]
    bkv_sb, bq64_sb, bvb_sb = consts["bkv"], consts["bq64"], consts["bvb"]
    ident, ident_a = consts["ident"], consts["ident_r"]

    with ExitStack() as ctx:
        persist = ctx.enter_context(tc.tile_pool(name=f"pp_persist{r}", bufs=1))
        xt_sb = persist.tile([128, 8, ET, 256], DTX, tag="xt", name=f"xt_{r}")
        kv_sb = persist.tile([128, S], DTA, tag="kv", name=f"kv_{r}")
        q_sb = persist.tile([H, HALF], DTA, tag="q", name=f"q_{r}")
        v_sb = [
            persist.tile([128, H + 1], DTA, tag=f"v{kt}", name=f"pv{kt}_{r}")
            for kt in range(KT)
        ]
        f_all = persist.tile([128, 8, H], F32, tag="f", name=f"pf_{r}")

        for ch in range(8):
            nc.sync.dma_start(out=xt_sb[:, ch], in_=xt4[ch])

        pA = ctx.enter_context(
            tc.tile_pool(name=f"ppA{r}", bufs=2, space="PSUM"))
        ps = ctx.enter_context(
            tc.tile_pool(name=f"pps{r}", bufs=2, space="PSUM"))
        po = ctx.enter_context(
            tc.tile_pool(name=f"ppo{r}", bufs=1, space="PSUM"))
        ppool = ctx.enter_context(tc.tile_pool(name=f"ppp{r}", bufs=8))
        opool = ctx.enter_context(tc.tile_pool(name=f"ppop{r}", bufs=4))

        def proj_kv(ch):
            ps_kv = pA.tile([128, 256], F32, tag="p", name=f"pskv{ch}_{r}")
            for et in range(ET):
                nc.tensor.matmul(ps_kv, wkv_sb[:, et], xt_sb[:, ch, et],
                                 start=(et == 0), stop=(et == ET - 1))
            nc.vector.tensor_scalar_add(
                kv_sb[:, ch * 256:(ch + 1) * 256], ps_kv, bkv_sb)

        def proj_q(ch):
            ps_q = pA.tile([H, 256], F32, tag="p", name=f"psq{ch}_{r}")
            for et in range(ET):
                nc.tensor.matmul(ps_q, wq_sb[:, et], xt_sb[:, ch, et],
                                 start=(et == 0), stop=(et == ET - 1))
            nc.vector.tensor_scalar_add(
                q_sb[:, ch * 256:(ch + 1) * 256], ps_q, bq64_sb)

        def v_tr(kt):
            pst = pA.tile([128, H + 1], DTA, tag="p", name=f"pst{kt}_{r}")
            nc.tensor.transpose(
                pst[:, 0:H], kv_sb[64:128, kt * 128:(kt + 1) * 128],
                ident_a[64:128, 64:128])
            nc.vector.tensor_copy(v_sb[kt][:, 0:H], pst[:, 0:H])
            ones_col = v_sb[kt][:, H:H + 1]
            nc.vector.memset(
                ones_col.bitcast(F32)
                if DTA == mybir.dt.float32r else ones_col, 1.0)

        o_ps = po.tile([H + 1, 2, 512], F32, tag="o", name=f"po_{r}")

        def attn_pair(g, j):
            # k-tiles 2g, 2g+1 against query half j: two S matmuls, one
            # paired exp, two PV accumulations.
            s_ps = ps.tile([128, 2, 512], F32, tag="s", name=f"pss{g}_{j}_{r}")
            qv = q_sb[:, j * 512:(j + 1) * 512]
            for i in range(2):
                kt = 2 * g + i
                nc.tensor.matmul(s_ps[:, i], kv_sb[0:64, kt * 128:(kt + 1) * 128],
                                 qv, start=True, stop=True)
            p_t = ppool.tile([128, 2, 512], DTA, tag="p", name=f"ppt{g}_{j}_{r}")
            nc.scalar.activation(
                p_t, s_ps, mybir.ActivationFunctionType.Exp, scale=SCALE)
            for i in range(2):
                kt = 2 * g + i
                nc.tensor.matmul(o_ps[:, j], v_sb[kt], p_t[:, i],
                                 start=(kt == 0), stop=(kt == KT - 1))

        def phase_c(qb):
            # one bulk eviction of the PV accumulator, then the four
            # 128-query transposes run back-to-back
            o_t = opool.tile([H + 1, 512], F32, tag="ot",
                             name=f"pot{qb}_{r}")
            nc.vector.tensor_copy(o_t, o_ps[:, qb])
            for sub in range(4):
                ps2 = pA.tile([128, H + 1], F32, tag="p",
                              name=f"pps2_{qb}{sub}_{r}")
                nc.tensor.transpose(
                    ps2, o_t[:, sub * 128:(sub + 1) * 128],
                    ident[0:H + 1, 0:H + 1])
                r_t = opool.tile([128, 1], F32, tag="rt",
                                 name=f"prt{qb}{sub}_{r}")
                nc.vector.reciprocal(r_t, ps2[:, H:H + 1])
                fa = f_all[:, qb * 4 + sub, :]
                nc.vector.scalar_tensor_tensor(
                    fa, ps2[:, 0:H], r_t, bvb_sb,
                    mybir.AluOpType.mult, mybir.AluOpType.add)
            nc.sync.dma_start(
                out=out[:, qb * 4:(qb + 1) * 4, :],
                in_=f_all[:, qb * 4:(qb + 1) * 4, :])

        proj_kv(0)
        proj_q(0)
        proj_kv(1)
        proj_q(1)
        for kt in range(4):
            v_tr(kt)
        # stream j=0 (first 512 queries) chases the kv projections
        attn_pair(0, 0)
        proj_kv(2)
        proj_kv(3)
        for kt in range(4, 8):
            v_tr(kt)
        attn_pair(1, 0)
        proj_q(2)
        proj_q(3)
        for g in range(2, 4):
            attn_pair(g, 0)
        proj_kv(4)
        proj_kv(5)
        for kt in range(8, 12):
            v_tr(kt)
        proj_kv(6)
        proj_kv(7)
        for kt in range(12, 16):
            v_tr(kt)
        for g in range(4, 8):
            attn_pair(g, 0)
        # stream j=1; qb0's phase C + output overlap its back half
        for g in range(5):
            attn_pair(g, 1)
        phase_c(0)
        for g in range(5, 8):
            attn_pair(g, 1)
        phase_c(1)


def build(mm_mode: str = "f32r", reps: int = 1, upto: str = "full", layout: str = "dup"):
    """Builds + compiles the SPMD single-core program. Returns nc."""
    DT = {"f32r": mybir.dt.float32r, "f32": F32, "mix": mybir.dt.bfloat16,
          "bf16": mybir.dt.bfloat16}[mm_mode]

    nc = bacc.Bacc("TRN2", target_bir_lowering=False)

    if layout == "pipe":
        DTX, DTA = {
            "bf16": (mybir.dt.bfloat16, mybir.dt.bfloat16),
            "f32r": (mybir.dt.float32r, mybir.dt.float32r),
            "mix": (mybir.dt.bfloat16, mybir.dt.float32r),
        }[mm_mode]
        dram = {
            "xt4": nc.dram_tensor("xt4", [8, 128, ET, 256], DTX,
                                  kind="ExternalInput"),
            "wkv2": nc.dram_tensor("wkv2", [128, ET, 128], DTX,
                                   kind="ExternalInput"),
            "wq": nc.dram_tensor("wq", [128, ET, H], DTX,
                                 kind="ExternalInput"),
            "bkv": nc.dram_tensor("bkv", [128, 1], F32, kind="ExternalInput"),
            "bq64": nc.dram_tensor("bq64", [H, 1], F32, kind="ExternalInput"),
            "bvb": nc.dram_tensor("bvb", [128, H], F32, kind="ExternalInput"),
            "out": nc.dram_tensor("out", [128, 8, H], F32,
                                  kind="ExternalOutput"),
        }
        with tile.TileContext(nc) as tc, ExitStack() as ctx:
            cp = ctx.enter_context(tc.tile_pool(name="consts", bufs=1))
            ident = cp.tile([128, 128], F32, tag="ident")
            make_identity(nc, ident)
            ident_r = cp.tile([128, 128], DTA, tag="ident_r")
            nc.vector.tensor_copy(ident_r, ident)
            wkv_sb = cp.tile([128, ET, 128], DTX, tag="wkv2")
            nc.sync.dma_start(out=wkv_sb, in_=dram["wkv2"][:])
            wq_sb = cp.tile([128, ET, H], DTX, tag="wq")
            nc.scalar.dma_start(out=wq_sb, in_=dram["wq"][:])
            bkv_sb = cp.tile([128, 1], F32, tag="bkv")
            nc.scalar.dma_start(out=bkv_sb, in_=dram["bkv"][:])
            bq64_sb = cp.tile([H, 1], F32, tag="bq64")
            nc.scalar.dma_start(out=bq64_sb, in_=dram["bq64"][:])
            bvb_sb = cp.tile([128, H], F32, tag="bvb")
            nc.scalar.dma_start(out=bvb_sb, in_=dram["bvb"][:])
            consts = dict(wkv2=wkv_sb, wq=wq_sb, bkv=bkv_sb, bq64=bq64_sb,
                          bvb=bvb_sb, ident=ident, ident_r=ident_r)
            for r in range(reps):
                _emit_pipe(nc, tc, DTX, DTA, dram, consts, r)
        nc.compile()
        return nc

    dram = {
        "xt": nc.dram_tensor("xt", [2, 128, 4, ET * HALF // 4], DT, kind="ExternalInput"),
        "wvq": nc.dram_tensor("wvq", [128, ET, 128], DT, kind="ExternalInput"),
        "wk": nc.dram_tensor("wk", [128, ET, 128], DT, kind="ExternalInput"),
        "wvk": nc.dram_tensor("wvk", [128, ET, 128], DT, kind="ExternalInput"),
        "b0q": nc.dram_tensor("b0q", [128, 1], F32, kind="ExternalInput"),
        "b0k": nc.dram_tensor("b0k", [128, 1], F32, kind="ExternalInput"),
        "bvb": nc.dram_tensor("bvb", [128, H], F32, kind="ExternalInput"),
        "wkv2": nc.dram_tensor("wkv2", [128, ET, 128], DT, kind="ExternalInput"),
        "wq": nc.dram_tensor("wq", [128, ET, H], DT, kind="ExternalInput"),
        "bkv": nc.dram_tensor("bkv", [128, 1], F32, kind="ExternalInput"),
        "bq64": nc.dram_tensor("bq64", [H, 1], F32, kind="ExternalInput"),
        "out": nc.dram_tensor("out", [HALF, H], F32, kind="ExternalOutput"),
    }

    with tile.TileContext(nc) as tc, ExitStack() as ctx:
        cp = ctx.enter_context(tc.tile_pool(name="consts", bufs=1))
        consts = {}
        wvq_sb = cp.tile([128, ET, 128], DT, tag="wvq")
        nc.sync.dma_start(out=wvq_sb, in_=dram["wvq"][:])
        wk_sb = cp.tile([128, ET, 128], DT, tag="wk")
        nc.sync.dma_start(out=wk_sb, in_=dram["wk"][:])
        wvk_sb = cp.tile([128, ET, 128], DT, tag="wvk")
        nc.sync.dma_start(out=wvk_sb, in_=dram["wvk"][:])
        b0q_sb = cp.tile([128, 1], F32, tag="b0q")
        nc.sync.dma_start(out=b0q_sb, in_=dram["b0q"][:])
        b0k_sb = cp.tile([128, 1], F32, tag="b0k")
        nc.sync.dma_start(out=b0k_sb, in_=dram["b0k"][:])
        bvb_sb = cp.tile([128, H], F32, tag="bvb")
        nc.sync.dma_start(out=bvb_sb, in_=dram["bvb"][:])
        ident = cp.tile([128, 128], F32, tag="ident")
        make_identity(nc, ident)
        ident_r = cp.tile([128, 128], DT, tag="ident_r")
        nc.vector.tensor_copy(ident_r, ident)
        if layout in ("cc", "cc2"):
            wkv2_sb = cp.tile([128, ET, 128], DT, tag="wkv2")
            nc.sync.dma_start(out=wkv2_sb, in_=dram["wkv2"][:])
            wq_sb = cp.tile([128, ET, H], DT, tag="wq")
            nc.sync.dma_start(out=wq_sb, in_=dram["wq"][:])
            bkv_sb = cp.tile([128, 1], F32, tag="bkv")
            nc.sync.dma_start(out=bkv_sb, in_=dram["bkv"][:])
            bq64_sb = cp.tile([H, 1], F32, tag="bq64")
            nc.sync.dma_start(out=bq64_sb, in_=dram["bq64"][:])
            consts.update(wkv2=wkv2_sb, wq=wq_sb, bkv=bkv_sb, bq64=bq64_sb)
        consts.update(
            wvq=wvq_sb, wk=wk_sb, wvk=wvk_sb, b0q=b0q_sb, b0k=b0k_sb,
            bvb=bvb_sb, ident=ident, ident_r=ident_r,
        )

        for r in range(reps):
            _emit_rep(nc, tc, DT, dram, consts, r, upto, layout)

    nc.compile()
    return nc


def shard_inputs(x, Wq, bq, Wk, bk, Wv, bv, mm_mode="f32r", layout=None):
    """Builds the per-core input maps (host-side layout prep).

    layout="pipe" builds only the tensors the pipe NEFF consumes (about
    half the numpy work); the default builds every layout's tensors."""
    if mm_mode in ("bf16", "mix"):
        import ml_dtypes
        dt_np = ml_dtypes.bfloat16
    else:
        dt_np = np.float32
    x = np.asarray(x, dtype=np.float32).astype(dt_np)
    Wq, Wk, Wv = (np.asarray(a, np.float32).astype(dt_np) for a in (Wq, Wk, Wv))
    bq, bk, bv = (np.asarray(a, np.float32) for a in (bq, bk, bv))
    z = np.zeros(64, np.float32)
    zw = np.zeros((E, 0), dtype=dt_np)

    def eperm(w):  # [E, d] -> [128, ET, d] with row (p, t) = w[8p + t]
        return np.ascontiguousarray(w.reshape(128, ET, -1))

    wkv2 = eperm(np.concatenate([Wk, Wv], axis=1))
    wqp = eperm(Wq)
    bkv = np.concatenate([bk, z])[:, None].copy()
    bq64 = bq[:, None].copy()
    bvb = np.ascontiguousarray(np.broadcast_to(bv, (128, H)))
    pipe_only = layout == "pipe"
    if not pipe_only:
        wvq = eperm(np.concatenate([Wv, Wq], axis=1))
        wvk = eperm(np.concatenate([Wv, Wk], axis=1))
        wkk = eperm(np.concatenate([Wk, Wk], axis=1))
        b0q = np.concatenate([z, bq])[:, None].copy()
        b0k = np.concatenate([z, bk])[:, None].copy()
    in_maps = []
    for c in range(NCORES):
        b, h = divmod(c, 2)
        own = x[b, h * HALF:(h + 1) * HALF].T        # [E, 1024]
        oth = x[b, (1 - h) * HALF:(2 - h) * HALF].T  # [E, 1024]
        # pipe layout: 8 x 256-token chunks (own x4 then oth x4), each
        # [128, ET, 256] with (p, et) holding e-row 8p+et.
        xt4 = np.ascontiguousarray(
            np.stack([own[:, i * 256:(i + 1) * 256] for i in range(4)]
                     + [oth[:, i * 256:(i + 1) * 256] for i in range(4)]
                     ).reshape(8, 128, ET, 256))
        m = {"xt4": xt4, "wkv2": wkv2, "wq": wqp, "bkv": bkv,
             "bq64": bq64, "bvb": bvb}
        if not pipe_only:
            # [2, 128, 4, 2048]: (half, p, chunk, j): e-row 8p+(chunk*2+j//1024)
            xt = np.stack([own, oth]).reshape(2, 128, 8, HALF)
            xt = np.ascontiguousarray(xt.reshape(2, 128, 4, ET * HALF // 4))
            m.update({"xt": xt, "wvq": wvq, "wk": wkk, "wvk": wvk,
                      "b0q": b0q, "b0k": b0k})
        in_maps.append(m)
    return in_maps


def gather_outputs(results):
    out = np.empty((B, S, H), np.float32)
    for c in range(NCORES):
        b, h = divmod(c, 2)
        oc = results[c]["out"]
        if oc.shape == (128, 8, H):  # pipe: token t*128+p at [p, t]
            oc = np.transpose(oc, (1, 0, 2)).reshape(HALF, H)
        out[b, h * HALF:(h + 1) * HALF] = oc
    return out


_NC_CACHE = {}


def _get_nc(mm_mode="f32r", reps=1, upto="full", layout="dup"):
    key = (mm_mode, reps, upto, layout)
    if key not in _NC_CACHE:
        _NC_CACHE[key] = build(mm_mode, reps, upto, layout)
    return _NC_CACHE[key]


def run(inputs, mm_mode="f32r", layout="cc", **kw):
    from concourse.bass_utils import run_bass_kernel_spmd

    nc = _get_nc(mm_mode, layout=layout)
    in_maps = shard_inputs(**inputs, mm_mode=mm_mode, layout=layout)
    res = run_bass_kernel_spmd(nc, in_maps, core_ids=list(range(NCORES)), **kw)
    return gather_outputs(res.results), res


def _build_exec(nc, in_maps):
    """Builds a re-invokable (non-donating) sharded executable + device args.

    Mirrors bass2jax.run_bass_via_pjrt's multi-core path, but keeps the
    output buffers as ordinary (non-donated) device arrays so the same
    callable can be executed repeatedly for wall-clock timing.
    """
    import jax
    from jax.sharding import Mesh, PartitionSpec, NamedSharding
    from jax.experimental.shard_map import shard_map
    from concourse import mybir
    from concourse.bass2jax import (
        _bass_exec_p, partition_id_tensor, install_neuronx_cc_hook,
    )

    install_neuronx_cc_hook()
    partition_name = nc.partition_id_tensor.name if nc.partition_id_tensor else None
    in_names, out_names, out_avals, zero_outs = [], [], [], []
    for alloc in nc.m.functions[0].allocations:
        if not isinstance(alloc, mybir.MemoryLocationSet):
            continue
        name = alloc.memorylocations[0].name
        if alloc.kind == "ExternalInput":
            if name != partition_name:
                in_names.append(name)
        elif alloc.kind == "ExternalOutput":
            out_names.append(name)
            shape = tuple(alloc.tensor_shape)
            dtype = mybir.dt.np(alloc.dtype)
            out_avals.append(jax.core.ShapedArray(shape, dtype))
            zero_outs.append(np.zeros(shape, dtype))
    n_params = len(in_names)
    all_in_names = list(in_names) + list(out_names)
    if partition_name is not None:
        all_in_names.append(partition_name)

    def _body(*args):
        operands = list(args)
        if partition_name is not None:
            operands.append(partition_id_tensor())
        outs = _bass_exec_p.bind(
            *operands,
            out_avals=tuple(out_avals),
            in_names=tuple(all_in_names),
            out_names=tuple(out_names),
            lowering_input_output_aliases=(),
            sim_require_finite=True,
            sim_require_nnan=True,
            nc=nc,
        )
        return tuple(outs)

    n_cores = len(in_maps)
    devices = jax.devices()[:n_cores]
    mesh = Mesh(np.asarray(devices), ("core",))
    nin = n_params + len(out_names)
    sharded = jax.jit(
        shard_map(
            _body, mesh=mesh,
            in_specs=(PartitionSpec("core"),) * nin,
            out_specs=(PartitionSpec("core"),) * len(out_names),
            check_rep=False,
        ),
        keep_unused=True,
    )
    sh = NamedSharding(mesh, PartitionSpec("core"))
    dev_args = [
        jax.device_put(
            np.concatenate([np.asarray(m[i]) for m in in_maps], axis=0), sh
        )
        for i in in_names
    ] + [
        jax.device_put(
            np.zeros((n_cores * z.shape[0], *z.shape[1:]), z.dtype), sh
        )
        for z in zero_outs
    ]
    return sharded, dev_args, out_names, out_avals


def _exec_results(r, out_names, out_avals):
    out_arrs = [np.asarray(a) for a in r]
    return [
        {
            name: out_arrs[i].reshape(NCORES, *out_avals[i].shape)[c]
            for i, name in enumerate(out_names)
        }
        for c in range(NCORES)
    ]


def bench(inputs, mm_mode="f32r", iters=50, reps=1, upto="full", layout="dup",
          n_cores=NCORES):
    """Amortized wall-clock per-execution time over repeated runs."""
    import jax, time

    nc = _get_nc(mm_mode, reps, upto, layout)
    in_maps = shard_inputs(**inputs, mm_mode=mm_mode)[:n_cores]
    fn, dev_args, out_names, out_avals = _build_exec(nc, in_maps)
    r = fn(*dev_args)
    jax.block_until_ready(r)  # compile + warm
    t0 = time.perf_counter()
    for _ in range(iters):
        r = fn(*dev_args)
    jax.block_until_ready(r)
    dt = (time.perf_counter() - t0) / iters
    if n_cores != NCORES:
        return None, dt
    return gather_outputs(_exec_results(r, out_names, out_avals)), dt


def kernel(**inputs) -> np.ndarray:
    try:
        out, _ = run(inputs, mm_mode="bf16", layout="pipe")
    except Exception:
        # Fall back to the proven collective-free data-parallel layout.
        out, _ = run(inputs, mm_mode="f32r", layout="dup")
    return out



# revision 62
# speedup vs baseline: 1.0462x; 1.0462x over previous
"""Single-head attention (B=4, S=2048, E=1024, H=64) on 8 trn2 NeuronCores.

Sharding: core c -> batch b = c//2, query-half h = c%2.  Each core is
fully independent (no collectives -- a 0.5 MB pairwise AllGather costs
~2.5 ms on this axon fabric, dwarfing the whole kernel): it loads x[b]
(both halves, bf16), computes q for its own 1024 queries and k/v for
all 2048 keys, and writes its [1024, 64] output slice.

Primary layout "pipe" (bf16, collective-free, pipelined):
  - x arrives as 8 token-chunks of [128p, 8et, 256tok] (partition p,
    slot et hold embedding row 8p+et); projections chase the chunk
    DMAs (weights go first on the same sync ring so they can never
    queue behind the bulk x transfers).
  - Per chunk: kv psum [128,256] accumulates over 8 e-tiles
    (lhsT=[Wk|Wv]-tile, rhs=x-chunk); q likewise for own chunks.  DVE
    evictions add per-partition biases ([bk|0]; bv deferred to phase C).
  - v tiles: PE-transpose of kv rows 64:128 per k-tile, DVE-evicted
    with an appended ones column so the PV matmul also accumulates
    softmax row-sums.
  - Attention runs as two 512-query streams (j=0 then j=1), each one
    PV accumulation chain over the 16 k-tiles:
      S^T[k,q] = matmul(lhsT=k-tile[64,128], rhs=q^T[64,512]) -> psum
      P^T = exp(S^T/8), one ACT op per k-tile pair ([128,2,512];
      scores are small: no max subtraction needed)
      out'[65,512] += matmul(lhsT=[v|1][128,65], rhs=P^T[128,512])
    Stream j=0 closes early so its phase C + output DMA overlap
    stream j=1.
  - Phase C: one bulk DVE eviction of the PV accumulator, then four
    back-to-back PE transposes -> [128q, 65], DVE reciprocal of the
    row-sum column, fused (out * 1/sum) + bv via scalar_tensor_tensor.
  PSUM budget (8 banks): proj/transpose pool 2 + scores 2x2 + PV 2.
  Measured/simulated ~41 us/rep (PE-column-bound: projections 24.6k +
  attention 32.8k columns are the floor for this sharding); the cc
  baseline measured 2.47 ms/rep on the same harness.

Fallback layout "dup" (f32r) is the older collective-free design; "cc"
and "cc2" (sequence-parallel, pairwise collectives) are kept for
reference but are ~25x slower end-to-end on this fabric.
"""

import numpy as np
from contextlib import ExitStack

import concourse.bacc as bacc
import concourse.tile as tile
from concourse import mybir
from concourse.masks import make_identity

B, S, E, H = 4, 2048, 1024, 64
NCORES = 8
HALF = S // 2  # queries per core
ET = E // 128  # e-tiles
KT = S // 128  # k-tiles
F32 = mybir.dt.float32

SCALE = 0.125  # 1/sqrt(H)


def _emit_rep(nc, tc, DT, dram, consts, r, upto="full", layout="dup"):
    """One full iteration of the kernel body (DMA + all phases)."""
    xt, out = dram["xt"], dram["out"]
    wvq_sb, wk_sb, wvk_sb = consts["wvq"], consts["wk"], consts["wvk"]
    b0q_sb, b0k_sb, bvb_sb = consts["b0q"], consts["b0k"], consts["bvb"]
    ident, ident_r = consts["ident"], consts["ident_r"]

    with ExitStack() as ctx:
        persist = ctx.enter_context(tc.tile_pool(name=f"persist{r}", bufs=1))

        # --- x^T (e-permuted: partition p holds e-rows 8p..8p+7, one
        # 1024-col s-segment per e).  4 x 1MB DMA chunks per half. ---
        xt_own = persist.tile([128, ET * HALF], DT, tag="xt_own",
                              name=f"xt_own_{r}")
        xt_oth = persist.tile([128, ET * HALF], DT, tag="xt_oth",
                              name=f"xt_oth_{r}")
        CH = ET * HALF // 4  # 2048 elems per partition per chunk
        _engs = [nc.sync, nc.scalar, nc.gpsimd]
        for c in range(4):
            _engs[c % 3].dma_start(
                out=xt_own[:, c * CH:(c + 1) * CH], in_=xt[0, :, c, :])
        for c in range(4):
            _engs[(c + 1) % 3].dma_start(
                out=xt_oth[:, c * CH:(c + 1) * CH], in_=xt[1, :, c, :])

        def xo_slice(t, et, blk):
            o = et * HALF + blk * 512
            return t[:, o:o + 512]

        # --- projection outputs ---
        # vq_own rows 0:64 = v^T_own, rows 64:128 = q^T_own
        vq_own = persist.tile([128, HALF], DT, tag="vq_own", name=f"vq_own_{r}")
        # k_own rows 64:128 = k^T of own half
        k_own = persist.tile([128, HALF], DT, tag="k_own", name=f"k_own_{r}")
        # vk_oth rows 0:64 = v^T_oth, rows 64:128 = k^T_oth
        vk_oth = persist.tile([128, HALF], DT, tag="vk_oth", name=f"vk_oth_{r}")
        v_sb = [
            persist.tile([128, H + 1], DT, tag=f"v{kt}", name=f"v{kt}_{r}")
            for kt in range(KT)
        ]
        f_all = persist.tile(
            [128, 2 * HALF // 256, H], F32, tag="f_all", name=f"f_all_{r}"
        )

        if upto == "dma":
            # DMA only + one anchor matmul per half so DCE keeps the loads
            with tc.tile_pool(name=f"pad{r}", bufs=1, space="PSUM") as pad:
                psd = pad.tile([128, 512], F32, tag="d", name=f"psd_{r}")
                for et in range(ET):
                    nc.tensor.matmul(psd, wvq_sb[:, et], xo_slice(xt_own, et, 0),
                                     start=(et == 0), stop=(et == ET - 1))
                for et in range(ET):
                    nc.tensor.matmul(psd, wvq_sb[:, et], xo_slice(xt_oth, et, 0),
                                     start=(et == 0), stop=(et == ET - 1))
                nc.vector.tensor_scalar_add(vq_own[:, 0:512], psd, b0q_sb)
                nc.vector.memset(f_all, 0.0)
                nc.sync.dma_start(
                    out=out.rearrange("(t p) d -> p t d", p=128),
                    in_=f_all)
            return

        if layout == "cc2":
            # Like cc, but the pair exchange is AllReduce(add) + local
            # subtract (no rank-dependent slot selection), and attention over
            # the core's own 8 k-tiles starts before the exchange completes
            # (softmax accumulation is key-order invariant).
            kv_mine = persist.tile([128, HALF], DT, tag="kv_mine",
                                   name=f"kv_mine_{r}")
            kv_peer = persist.tile([128, HALF], DT, tag="kv_peer",
                                   name=f"kv_peer_{r}")
            q_own = persist.tile([H, HALF], DT, tag="q_own", name=f"q_own_{r}")
            with tc.tile_pool(name=f"pao{r}", bufs=1, space="PSUM") as pao:
                for blk in range(2):
                    ps_kv = pao.tile([128, 512], F32, tag=f"akv{blk}",
                                     name=f"akv{blk}_{r}")
                    ps_q = pao.tile([H, 512], F32, tag=f"aq{blk}",
                                    name=f"aq{blk}_{r}")
                    for et in range(ET):
                        st = dict(start=(et == 0), stop=(et == ET - 1))
                        xo = xo_slice(xt_own, et, blk)
                        nc.tensor.matmul(ps_kv, consts["wkv2"][:, et], xo, **st)
                        nc.tensor.matmul(ps_q, consts["wq"][:, et], xo, **st)
                    c0, c1 = blk * 512, (blk + 1) * 512
                    nc.vector.tensor_scalar_add(
                        kv_mine[:, c0:c1], ps_kv, consts["bkv"])
                    nc.vector.tensor_scalar_add(
                        q_own[:, c0:c1], ps_q, consts["bq64"])
            with tc.tile_pool(name=f"dr{r}", bufs=1, space="DRAM") as drp:
                kv_local = drp.tile([128, HALF], DT, tag="kvl",
                                    name=f"kv_local_{r}")
                kv_sum = drp.tile([128, HALF], DT, tag="kvs",
                                  name=f"kv_sum_{r}")
                nc.sync.dma_start(out=kv_local[:], in_=kv_mine)
                nc.gpsimd.collective_compute(
                    "AllReduce",
                    mybir.AluOpType.add,
                    replica_groups=[[0, 1], [2, 3], [4, 5], [6, 7]],
                    ins=[kv_local.opt()],
                    outs=[kv_sum.opt()],
                )
                kv_sum_sb = persist.tile([128, HALF], DT, tag="kv_sum_sb",
                                         name=f"kv_sum_sb_{r}")
                nc.scalar.dma_start(out=kv_sum_sb, in_=kv_sum[:])
                nc.vector.tensor_sub(kv_peer, kv_sum_sb, kv_mine)

            psS = ctx.enter_context(
                tc.tile_pool(name=f"psS{r}", bufs=2, space="PSUM"))
            psO = ctx.enter_context(
                tc.tile_pool(name=f"psO{r}", bufs=2, space="PSUM"))
            pt = ctx.enter_context(
                tc.tile_pool(name=f"pt{r}", bufs=2, space="PSUM"))
            ppool = ctx.enter_context(tc.tile_pool(name=f"pp{r}", bufs=3))
            opool = ctx.enter_context(tc.tile_pool(name=f"op{r}", bufs=4))

            def kvsrc(kt):
                t = kv_mine if kt < 8 else kv_peer
                kk = kt % 8
                return t, kk * 128, (kk + 1) * 128

            def v_transpose2(kt):
                t, a, b = kvsrc(kt)
                pst = pt.tile([128, H + 1], DT, tag="tr", name=f"pst{kt}_{r}")
                nc.tensor.transpose(
                    pst[:, 0:H], t[64:128, a:b], ident_r[64:128, 64:128])
                nc.vector.tensor_copy(v_sb[kt][:, 0:H], pst[:, 0:H])
                ones_col = v_sb[kt][:, H:H + 1]
                nc.vector.memset(
                    ones_col.bitcast(F32) if DT == mybir.dt.float32r
                    else ones_col, 1.0)

            for kt in range(8):
                v_transpose2(kt)

            for qb in range(HALF // 512):
                q_ap = q_own[:, qb * 512:(qb + 1) * 512]
                o_ps = psO.tile([H + 1, 512], F32, tag="o", name=f"o_ps{qb}_{r}")
                for g in range(KT // 2):
                    if qb == 0 and g == 4:
                        for kt in range(8, KT):
                            v_transpose2(kt)
                    s_ps = psS.tile([128, 2, 512], F32, tag="s",
                                    name=f"s_ps{qb}_{g}_{r}")
                    for j in range(2):
                        kt = g * 2 + j
                        t, a, b = kvsrc(kt)
                        nc.tensor.matmul(s_ps[:, j], t[0:64, a:b], q_ap,
                                         start=True, stop=True)
                    p_sb = ppool.tile([128, 2, 512], DT, tag="p",
                                      name=f"p_sb{qb}_{g}_{r}")
                    nc.scalar.activation(
                        p_sb, s_ps, mybir.ActivationFunctionType.Exp,
                        scale=SCALE)
                    for j in range(2):
                        kt = g * 2 + j
                        nc.tensor.matmul(
                            o_ps, v_sb[kt], p_sb[:, j],
                            start=(kt == 0), stop=(kt == KT - 1),
                        )
                for sub in range(4):
                    o_t = opool.tile([H + 1, 128], F32, tag="ot",
                                     name=f"ot{qb}{sub}_{r}")
                    nc.vector.tensor_copy(o_t, o_ps[:, sub * 128:(sub + 1) * 128])
                    ps2 = pt.tile([128, H + 1], F32, tag="tr",
                                  name=f"ps2_{qb}{sub}_{r}")
                    nc.tensor.transpose(ps2, o_t, ident[0:H + 1, 0:H + 1])
                    r_t = opool.tile([128, 1], F32, tag="rt",
                                     name=f"rt{qb}{sub}_{r}")
                    nc.vector.reciprocal(r_t, ps2[:, H:H + 1])
                    fa = f_all[:, qb * 4 + sub, :]
                    nc.vector.tensor_scalar_mul(fa, ps2[:, 0:H], r_t)
                    nc.vector.tensor_add(fa, fa, bvb_sb)

            nc.sync.dma_start(
                out=out.rearrange("(t p) d -> p t d", p=128), in_=f_all)
            return

        if layout == "cc":
            # Sequence-parallel projections: each core projects only its own
            # half (k^T rows 0:64 + bk, v^T rows 64:128, q^T separately),
            # then pairwise-AllGathers [k^T; v^T] so both cores hold the
            # full-sequence k/v.
            kv_mine = persist.tile([128, HALF], DT, tag="kv_mine",
                                   name=f"kv_mine_{r}")
            q_own = persist.tile([H, HALF], DT, tag="q_own", name=f"q_own_{r}")
            kv_all = persist.tile([128, 2, HALF], DT, tag="kv_all",
                                  name=f"kv_all_{r}")
            with tc.tile_pool(name=f"pao{r}", bufs=1, space="PSUM") as pao:
                for blk in range(2):
                    ps_kv = pao.tile([128, 512], F32, tag=f"akv{blk}",
                                     name=f"akv{blk}_{r}")
                    ps_q = pao.tile([H, 512], F32, tag=f"aq{blk}",
                                    name=f"aq{blk}_{r}")
                    for et in range(ET):
                        st = dict(start=(et == 0), stop=(et == ET - 1))
                        xo = xo_slice(xt_own, et, blk)
                        nc.tensor.matmul(ps_kv, consts["wkv2"][:, et], xo, **st)
                        nc.tensor.matmul(ps_q, consts["wq"][:, et], xo, **st)
                    c0, c1 = blk * 512, (blk + 1) * 512
                    nc.vector.tensor_scalar_add(
                        kv_mine[:, c0:c1], ps_kv, consts["bkv"])
                    nc.vector.tensor_scalar_add(
                        q_own[:, c0:c1], ps_q, consts["bq64"])
            with tc.tile_pool(name=f"dr{r}", bufs=1, space="DRAM") as drp:
                kv_local = drp.tile([128, HALF], DT, tag="kvl",
                                    name=f"kv_local_{r}")
                kv_pair = drp.tile([2, 128, HALF], DT, tag="kvp",
                                   name=f"kv_pair_{r}")
                nc.sync.dma_start(out=kv_local[:], in_=kv_mine)
                nc.gpsimd.collective_compute(
                    "AllGather",
                    mybir.AluOpType.bypass,
                    replica_groups=[[0, 1], [2, 3], [4, 5], [6, 7]],
                    ins=[kv_local.opt()],
                    outs=[kv_pair.opt()],
                )
                for s in range(2):
                    nc.sync.dma_start(out=kv_all[:, s, :], in_=kv_pair[s])

            psS = ctx.enter_context(
                tc.tile_pool(name=f"psS{r}", bufs=2, space="PSUM"))
            psO = ctx.enter_context(
                tc.tile_pool(name=f"psO{r}", bufs=1, space="PSUM"))
            pt = ctx.enter_context(
                tc.tile_pool(name=f"pt{r}", bufs=2, space="PSUM"))
            ppool = ctx.enter_context(tc.tile_pool(name=f"pp{r}", bufs=3))
            opool = ctx.enter_context(tc.tile_pool(name=f"op{r}", bufs=4))

            for kt in range(KT):
                slot, kk = kt // 8, kt % 8
                srcv = kv_all[64:128, slot, kk * 128:(kk + 1) * 128]
                pst = pt.tile([128, H + 1], DT, tag="tr", name=f"pst{kt}_{r}")
                nc.tensor.transpose(
                    pst[:, 0:H], srcv, ident_r[64:128, 64:128])
                nc.vector.tensor_copy(v_sb[kt][:, 0:H], pst[:, 0:H])
                ones_col = v_sb[kt][:, H:H + 1]
                nc.vector.memset(
                    ones_col.bitcast(F32) if DT == mybir.dt.float32r else ones_col,
                    1.0)

            for qb in range(HALF // 512):
                q_ap = q_own[:, qb * 512:(qb + 1) * 512]
                o_ps = psO.tile([H + 1, 512], F32, tag="o", name=f"o_ps{qb}_{r}")
                for g in range(KT // 2):
                    s_ps = psS.tile([128, 2, 512], F32, tag="s",
                                    name=f"s_ps{qb}_{g}_{r}")
                    for j in range(2):
                        kt = g * 2 + j
                        slot, kk = kt // 8, kt % 8
                        kl = kv_all[0:64, slot, kk * 128:(kk + 1) * 128]
                        nc.tensor.matmul(s_ps[:, j], kl, q_ap,
                                         start=True, stop=True)
                    p_sb = ppool.tile([128, 2, 512], DT, tag="p",
                                      name=f"p_sb{qb}_{g}_{r}")
                    nc.scalar.activation(
                        p_sb, s_ps, mybir.ActivationFunctionType.Exp,
                        scale=SCALE)
                    for j in range(2):
                        kt = g * 2 + j
                        nc.tensor.matmul(
                            o_ps, v_sb[kt], p_sb[:, j],
                            start=(kt == 0), stop=(kt == KT - 1),
                        )
                for sub in range(4):
                    o_t = opool.tile([H + 1, 128], F32, tag="ot",
                                     name=f"ot{qb}{sub}_{r}")
                    nc.vector.tensor_copy(o_t, o_ps[:, sub * 128:(sub + 1) * 128])
                    ps2 = pt.tile([128, H + 1], F32, tag="tr",
                                  name=f"ps2_{qb}{sub}_{r}")
                    nc.tensor.transpose(ps2, o_t, ident[0:H + 1, 0:H + 1])
                    r_t = opool.tile([128, 1], F32, tag="rt",
                                     name=f"rt{qb}{sub}_{r}")
                    nc.vector.reciprocal(r_t, ps2[:, H:H + 1])
                    fa = f_all[:, qb * 4 + sub, :]
                    nc.vector.tensor_scalar_mul(fa, ps2[:, 0:H], r_t)
                    nc.vector.tensor_add(fa, fa, bvb_sb)

            nc.sync.dma_start(
                out=out.rearrange("(t p) d -> p t d", p=128), in_=f_all)
            return

        # ---------- Phase A (own half): q^T, k^T_own, v^T_own ----------
        with tc.tile_pool(name=f"pao{r}", bufs=1, space="PSUM") as pao:
            ps_vq = [
                pao.tile([128, 512], F32, tag=f"avq{i}", name=f"avq{i}_{r}")
                for i in range(2)
            ]
            ps_k = [
                pao.tile([128, 512], F32, tag=f"ak{i}", name=f"ak{i}_{r}")
                for i in range(2)
            ]
            for et in range(ET):
                st = dict(start=(et == 0), stop=(et == ET - 1))
                for blk in range(2):
                    xo = xo_slice(xt_own, et, blk)
                    nc.tensor.matmul(ps_vq[blk], wvq_sb[:, et], xo, **st)
                    nc.tensor.matmul(ps_k[blk], wk_sb[:, et], xo, **st)
            for blk in range(2):
                c0, c1 = blk * 512, (blk + 1) * 512
                nc.vector.tensor_scalar_add(vq_own[:, c0:c1], ps_vq[blk], b0q_sb)
                nc.vector.tensor_scalar_add(
                    k_own[64:128, c0:c1], ps_k[blk][64:128, :], b0k_sb[64:128, :]
                )

        # Remaining PSUM budget (8 banks): psS 4 + psO 1 + pt 2 + pa2 1
        psS = ctx.enter_context(tc.tile_pool(name=f"psS{r}", bufs=2, space="PSUM"))
        psO = ctx.enter_context(tc.tile_pool(name=f"psO{r}", bufs=1, space="PSUM"))
        pt = ctx.enter_context(tc.tile_pool(name=f"pt{r}", bufs=2, space="PSUM"))
        pa2 = ctx.enter_context(tc.tile_pool(name=f"pa2{r}", bufs=1, space="PSUM"))
        ppool = ctx.enter_context(tc.tile_pool(name=f"pp{r}", bufs=3))
        opool = ctx.enter_context(tc.tile_pool(name=f"op{r}", bufs=4))

        def v_transpose(kt):
            src = (
                vq_own[0:64, kt * 128:(kt + 1) * 128]
                if kt < 8
                else vk_oth[0:64, (kt - 8) * 128:(kt - 7) * 128]
            )
            pst = pt.tile([128, H + 1], DT, tag="tr", name=f"pst{kt}_{r}")
            nc.tensor.transpose(pst[:, 0:H], src, ident_r[0:64, 0:64])
            nc.vector.tensor_copy(v_sb[kt][:, 0:H], pst[:, 0:H])
            ones_col = v_sb[kt][:, H:H + 1]
            nc.vector.memset(
                ones_col.bitcast(F32) if DT == mybir.dt.float32r else ones_col,
                1.0)

        # ---------- Phase A2 (own half v tiles) ----------
        for kt in range(8):
            v_transpose(kt)

        # ---------- Phase A (other half): k^T_oth, v^T_oth ----------
        for blk in range(2):
            ps_vk = pa2.tile([128, 512], F32, tag="avk", name=f"avk{blk}_{r}")
            for et in range(ET):
                xf = xo_slice(xt_oth, et, blk)
                nc.tensor.matmul(
                    ps_vk, wvk_sb[:, et], xf,
                    start=(et == 0), stop=(et == ET - 1),
                )
            nc.vector.tensor_scalar_add(
                vk_oth[:, blk * 512:(blk + 1) * 512], ps_vk, b0k_sb
            )
        for kt in range(8, KT):
            v_transpose(kt)

        if upto == "proj":
            nc.vector.memset(f_all, 0.0)
            nc.sync.dma_start(
                out=out.rearrange("(t p) d -> p t d", p=128), in_=f_all)
            return

        # ---------- Phase B + C: attention per query block ----------
        for qb in range(HALF // 512):
            q_ap = vq_own[64:128, qb * 512:(qb + 1) * 512]
            o_ps = psO.tile([H + 1, 512], F32, tag="o", name=f"o_ps{qb}_{r}")
            for g in range(KT // 2):
                s_ps = psS.tile([128, 2, 512], F32, tag="s", name=f"s_ps{qb}_{g}_{r}")
                for j in range(2):
                    kt = g * 2 + j
                    kl = (
                        k_own[64:128, kt * 128:(kt + 1) * 128]
                        if kt < 8
                        else vk_oth[64:128, (kt - 8) * 128:(kt - 7) * 128]
                    )
                    nc.tensor.matmul(s_ps[:, j], kl, q_ap, start=True, stop=True)
                p_sb = ppool.tile([128, 2, 512], DT, tag="p", name=f"p_sb{qb}_{g}_{r}")
                nc.scalar.activation(
                    p_sb, s_ps, mybir.ActivationFunctionType.Exp, scale=SCALE
                )
                for j in range(2):
                    kt = g * 2 + j
                    nc.tensor.matmul(
                        o_ps, v_sb[kt], p_sb[:, j],
                        start=(kt == 0), stop=(kt == KT - 1),
                    )
            if upto == "attn":
                nc.vector.tensor_copy(f_all[0:65, qb * 4, :], o_ps[:, 0:64])
                continue
            for sub in range(4):
                o_t = opool.tile([H + 1, 128], F32, tag="ot", name=f"ot{qb}{sub}_{r}")
                nc.vector.tensor_copy(o_t, o_ps[:, sub * 128:(sub + 1) * 128])
                ps2 = pt.tile([128, H + 1], F32, tag="tr", name=f"ps2_{qb}{sub}_{r}")
                nc.tensor.transpose(ps2, o_t, ident[0:H + 1, 0:H + 1])
                r_t = opool.tile([128, 1], F32, tag="rt", name=f"rt{qb}{sub}_{r}")
                nc.vector.reciprocal(r_t, ps2[:, H:H + 1])
                fa = f_all[:, qb * 4 + sub, :]
                nc.vector.tensor_scalar_mul(fa, ps2[:, 0:H], r_t)
                nc.vector.tensor_add(fa, fa, bvb_sb)

        nc.sync.dma_start(out=out.rearrange("(t p) d -> p t d", p=128), in_=f_all)


def _emit_pipe(nc, tc, DTX, DTA, dram, consts, r):
    """dup-style (collective-free) pipelined rep: token-chunked x DMA
    chased by projections; attention over own k-tiles interleaves with
    the other-half projections; single PV accumulation pair per core.

    DTX: dtype of x + projection weights (bf16 halves DMA but forces an
    InstLdweights per projection matmul).  DTA: dtype of kv/q/v/p SBUF
    storage (f32r keeps attention matmuls self-loading: no Ldweights).

    Per core: 1024 own queries, full 2048 keys.  PSUM budget (8 banks):
    pA 2 (proj kv/q + transposes) + ps 2x2 (scores pairs) + po 2 (PV).
    """
    xt4, out = dram["xt4"], dram["out"]
    wkv_sb, wq_sb = consts["wkv2"], consts["wq"]
    bkv_sb, bq64_sb, bvb_sb = consts["bkv"], consts["bq64"], consts["bvb"]
    ident, ident_a = consts["ident"], consts["ident_r"]

    with ExitStack() as ctx:
        persist = ctx.enter_context(tc.tile_pool(name=f"pp_persist{r}", bufs=1))
        xt_sb = persist.tile([128, 8, ET, 256], DTX, tag="xt", name=f"xt_{r}")
        kv_sb = persist.tile([128, S], DTA, tag="kv", name=f"kv_{r}")
        q_sb = persist.tile([H, HALF], DTA, tag="q", name=f"q_{r}")
        v_sb = [
            persist.tile([128, H + 1], DTA, tag=f"v{kt}", name=f"pv{kt}_{r}")
            for kt in range(KT)
        ]
        f_all = persist.tile([128, 8, H], F32, tag="f", name=f"pf_{r}")

        for ch in range(8):
            nc.sync.dma_start(out=xt_sb[:, ch], in_=xt4[ch])

        pA = ctx.enter_context(
            tc.tile_pool(name=f"ppA{r}", bufs=2, space="PSUM"))
        ps = ctx.enter_context(
            tc.tile_pool(name=f"pps{r}", bufs=2, space="PSUM"))
        po = ctx.enter_context(
            tc.tile_pool(name=f"ppo{r}", bufs=1, space="PSUM"))
        ppool = ctx.enter_context(tc.tile_pool(name=f"ppp{r}", bufs=8))
        opool = ctx.enter_context(tc.tile_pool(name=f"ppop{r}", bufs=4))

        def proj_kv(ch):
            ps_kv = pA.tile([128, 256], F32, tag="p", name=f"pskv{ch}_{r}")
            for et in range(ET):
                nc.tensor.matmul(ps_kv, wkv_sb[:, et], xt_sb[:, ch, et],
                                 start=(et == 0), stop=(et == ET - 1))
            nc.vector.tensor_scalar_add(
                kv_sb[:, ch * 256:(ch + 1) * 256], ps_kv, bkv_sb)

        def proj_q(ch):
            ps_q = pA.tile([H, 256], F32, tag="p", name=f"psq{ch}_{r}")
            for et in range(ET):
                nc.tensor.matmul(ps_q, wq_sb[:, et], xt_sb[:, ch, et],
                                 start=(et == 0), stop=(et == ET - 1))
            nc.vector.tensor_scalar_add(
                q_sb[:, ch * 256:(ch + 1) * 256], ps_q, bq64_sb)

        def v_tr(kt):
            pst = pA.tile([128, H + 1], DTA, tag="p", name=f"pst{kt}_{r}")
            nc.tensor.transpose(
                pst[:, 0:H], kv_sb[64:128, kt * 128:(kt + 1) * 128],
                ident_a[64:128, 64:128])
            nc.vector.tensor_copy(v_sb[kt][:, 0:H], pst[:, 0:H])
            ones_col = v_sb[kt][:, H:H + 1]
            nc.vector.memset(
                ones_col.bitcast(F32)
                if DTA == mybir.dt.float32r else ones_col, 1.0)

        o_ps = po.tile([H + 1, 2, 512], F32, tag="o", name=f"po_{r}")

        def attn_pair(g, j):
            # k-tiles 2g, 2g+1 against query half j: two S matmuls, one
            # paired exp, two PV accumulations.
            s_ps = ps.tile([128, 2, 512], F32, tag="s", name=f"pss{g}_{j}_{r}")
            qv = q_sb[:, j * 512:(j + 1) * 512]
            for i in range(2):
                kt = 2 * g + i
                nc.tensor.matmul(s_ps[:, i], kv_sb[0:64, kt * 128:(kt + 1) * 128],
                                 qv, start=True, stop=True)
            p_t = ppool.tile([128, 2, 512], DTA, tag="p", name=f"ppt{g}_{j}_{r}")
            nc.scalar.activation(
                p_t, s_ps, mybir.ActivationFunctionType.Exp, scale=SCALE)
            for i in range(2):
                kt = 2 * g + i
                nc.tensor.matmul(o_ps[:, j], v_sb[kt], p_t[:, i],
                                 start=(kt == 0), stop=(kt == KT - 1))

        def phase_c(qb):
            # one bulk eviction of the PV accumulator, then the four
            # 128-query transposes run back-to-back
            o_t = opool.tile([H + 1, 512], F32, tag="ot",
                             name=f"pot{qb}_{r}")
            nc.vector.tensor_copy(o_t, o_ps[:, qb])
            for sub in range(4):
                ps2 = pA.tile([128, H + 1], F32, tag="p",
                              name=f"pps2_{qb}{sub}_{r}")
                nc.tensor.transpose(
                    ps2, o_t[:, sub * 128:(sub + 1) * 128],
                    ident[0:H + 1, 0:H + 1])
                r_t = opool.tile([128, 1], F32, tag="rt",
                                 name=f"prt{qb}{sub}_{r}")
                nc.vector.reciprocal(r_t, ps2[:, H:H + 1])
                fa = f_all[:, qb * 4 + sub, :]
                nc.vector.scalar_tensor_tensor(
                    fa, ps2[:, 0:H], r_t, bvb_sb,
                    mybir.AluOpType.mult, mybir.AluOpType.add)
            nc.sync.dma_start(
                out=out[:, qb * 4:(qb + 1) * 4, :],
                in_=f_all[:, qb * 4:(qb + 1) * 4, :])

        proj_kv(0)
        proj_q(0)
        proj_kv(1)
        proj_q(1)
        for kt in range(4):
            v_tr(kt)
        # stream j=0 (first 512 queries) chases the kv projections
        attn_pair(0, 0)
        proj_kv(2)
        proj_kv(3)
        for kt in range(4, 8):
            v_tr(kt)
        attn_pair(1, 0)
        proj_q(2)
        proj_q(3)
        for g in range(2, 4):
            attn_pair(g, 0)
        proj_kv(4)
        proj_kv(5)
        for kt in range(8, 12):
            v_tr(kt)
        proj_kv(6)
        proj_kv(7)
        for kt in range(12, 16):
            v_tr(kt)
        for g in range(4, 8):
            attn_pair(g, 0)
        # stream j=1; qb0's phase C + output overlap its back half
        for g in range(5):
            attn_pair(g, 1)
        phase_c(0)
        for g in range(5, 8):
            attn_pair(g, 1)
        phase_c(1)


def build(mm_mode: str = "f32r", reps: int = 1, upto: str = "full", layout: str = "dup"):
    """Builds + compiles the SPMD single-core program. Returns nc."""
    DT = {"f32r": mybir.dt.float32r, "f32": F32, "mix": mybir.dt.bfloat16,
          "bf16": mybir.dt.bfloat16}[mm_mode]

    nc = bacc.Bacc("TRN2", target_bir_lowering=False)

    if layout == "pipe":
        DTX, DTA = {
            "bf16": (mybir.dt.bfloat16, mybir.dt.bfloat16),
            "f32r": (mybir.dt.float32r, mybir.dt.float32r),
            "mix": (mybir.dt.bfloat16, mybir.dt.float32r),
        }[mm_mode]
        dram = {
            "xt4": nc.dram_tensor("xt4", [8, 128, ET, 256], DTX,
                                  kind="ExternalInput"),
            "wkv2": nc.dram_tensor("wkv2", [128, ET, 128], DTX,
                                   kind="ExternalInput"),
            "wq": nc.dram_tensor("wq", [128, ET, H], DTX,
                                 kind="ExternalInput"),
            "bkv": nc.dram_tensor("bkv", [128, 1], F32, kind="ExternalInput"),
            "bq64": nc.dram_tensor("bq64", [H, 1], F32, kind="ExternalInput"),
            "bvb": nc.dram_tensor("bvb", [128, H], F32, kind="ExternalInput"),
            "out": nc.dram_tensor("out", [128, 8, H], F32,
                                  kind="ExternalOutput"),
        }
        with tile.TileContext(nc) as tc, ExitStack() as ctx:
            cp = ctx.enter_context(tc.tile_pool(name="consts", bufs=1))
            ident = cp.tile([128, 128], F32, tag="ident")
            make_identity(nc, ident)
            ident_r = cp.tile([128, 128], DTA, tag="ident_r")
            nc.vector.tensor_copy(ident_r, ident)
            wkv_sb = cp.tile([128, ET, 128], DTX, tag="wkv2")
            nc.sync.dma_start(out=wkv_sb, in_=dram["wkv2"][:])
            wq_sb = cp.tile([128, ET, H], DTX, tag="wq")
            nc.scalar.dma_start(out=wq_sb, in_=dram["wq"][:])
            bkv_sb = cp.tile([128, 1], F32, tag="bkv")
            nc.scalar.dma_start(out=bkv_sb, in_=dram["bkv"][:])
            bq64_sb = cp.tile([H, 1], F32, tag="bq64")
            nc.scalar.dma_start(out=bq64_sb, in_=dram["bq64"][:])
            bvb_sb = cp.tile([128, H], F32, tag="bvb")
            nc.scalar.dma_start(out=bvb_sb, in_=dram["bvb"][:])
            consts = dict(wkv2=wkv_sb, wq=wq_sb, bkv=bkv_sb, bq64=bq64_sb,
                          bvb=bvb_sb, ident=ident, ident_r=ident_r)
            for r in range(reps):
                _emit_pipe(nc, tc, DTX, DTA, dram, consts, r)
        nc.compile()
        return nc

    dram = {
        "xt": nc.dram_tensor("xt", [2, 128, 4, ET * HALF // 4], DT, kind="ExternalInput"),
        "wvq": nc.dram_tensor("wvq", [128, ET, 128], DT, kind="ExternalInput"),
        "wk": nc.dram_tensor("wk", [128, ET, 128], DT, kind="ExternalInput"),
        "wvk": nc.dram_tensor("wvk", [128, ET, 128], DT, kind="ExternalInput"),
        "b0q": nc.dram_tensor("b0q", [128, 1], F32, kind="ExternalInput"),
        "b0k": nc.dram_tensor("b0k", [128, 1], F32, kind="ExternalInput"),
        "bvb": nc.dram_tensor("bvb", [128, H], F32, kind="ExternalInput"),
        "wkv2": nc.dram_tensor("wkv2", [128, ET, 128], DT, kind="ExternalInput"),
        "wq": nc.dram_tensor("wq", [128, ET, H], DT, kind="ExternalInput"),
        "bkv": nc.dram_tensor("bkv", [128, 1], F32, kind="ExternalInput"),
        "bq64": nc.dram_tensor("bq64", [H, 1], F32, kind="ExternalInput"),
        "out": nc.dram_tensor("out", [HALF, H], F32, kind="ExternalOutput"),
    }

    with tile.TileContext(nc) as tc, ExitStack() as ctx:
        cp = ctx.enter_context(tc.tile_pool(name="consts", bufs=1))
        consts = {}
        wvq_sb = cp.tile([128, ET, 128], DT, tag="wvq")
        nc.sync.dma_start(out=wvq_sb, in_=dram["wvq"][:])
        wk_sb = cp.tile([128, ET, 128], DT, tag="wk")
        nc.sync.dma_start(out=wk_sb, in_=dram["wk"][:])
        wvk_sb = cp.tile([128, ET, 128], DT, tag="wvk")
        nc.sync.dma_start(out=wvk_sb, in_=dram["wvk"][:])
        b0q_sb = cp.tile([128, 1], F32, tag="b0q")
        nc.sync.dma_start(out=b0q_sb, in_=dram["b0q"][:])
        b0k_sb = cp.tile([128, 1], F32, tag="b0k")
        nc.sync.dma_start(out=b0k_sb, in_=dram["b0k"][:])
        bvb_sb = cp.tile([128, H], F32, tag="bvb")
        nc.sync.dma_start(out=bvb_sb, in_=dram["bvb"][:])
        ident = cp.tile([128, 128], F32, tag="ident")
        make_identity(nc, ident)
        ident_r = cp.tile([128, 128], DT, tag="ident_r")
        nc.vector.tensor_copy(ident_r, ident)
        if layout in ("cc", "cc2"):
            wkv2_sb = cp.tile([128, ET, 128], DT, tag="wkv2")
            nc.sync.dma_start(out=wkv2_sb, in_=dram["wkv2"][:])
            wq_sb = cp.tile([128, ET, H], DT, tag="wq")
            nc.sync.dma_start(out=wq_sb, in_=dram["wq"][:])
            bkv_sb = cp.tile([128, 1], F32, tag="bkv")
            nc.sync.dma_start(out=bkv_sb, in_=dram["bkv"][:])
            bq64_sb = cp.tile([H, 1], F32, tag="bq64")
            nc.sync.dma_start(out=bq64_sb, in_=dram["bq64"][:])
            consts.update(wkv2=wkv2_sb, wq=wq_sb, bkv=bkv_sb, bq64=bq64_sb)
        consts.update(
            wvq=wvq_sb, wk=wk_sb, wvk=wvk_sb, b0q=b0q_sb, b0k=b0k_sb,
            bvb=bvb_sb, ident=ident, ident_r=ident_r,
        )

        for r in range(reps):
            _emit_rep(nc, tc, DT, dram, consts, r, upto, layout)

    nc.compile()
    return nc


def shard_inputs(x, Wq, bq, Wk, bk, Wv, bv, mm_mode="f32r", layout=None):
    """Builds the per-core input maps (host-side layout prep).

    layout="pipe" builds only the tensors the pipe NEFF consumes (about
    half the numpy work); the default builds every layout's tensors."""
    if mm_mode in ("bf16", "mix"):
        import ml_dtypes
        dt_np = ml_dtypes.bfloat16
    else:
        dt_np = np.float32
    x = np.asarray(x, dtype=np.float32).astype(dt_np)
    Wq, Wk, Wv = (np.asarray(a, np.float32).astype(dt_np) for a in (Wq, Wk, Wv))
    bq, bk, bv = (np.asarray(a, np.float32) for a in (bq, bk, bv))
    z = np.zeros(64, np.float32)
    zw = np.zeros((E, 0), dtype=dt_np)

    def eperm(w):  # [E, d] -> [128, ET, d] with row (p, t) = w[8p + t]
        return np.ascontiguousarray(w.reshape(128, ET, -1))

    wkv2 = eperm(np.concatenate([Wk, Wv], axis=1))
    wqp = eperm(Wq)
    bkv = np.concatenate([bk, z])[:, None].copy()
    bq64 = bq[:, None].copy()
    bvb = np.ascontiguousarray(np.broadcast_to(bv, (128, H)))
    pipe_only = layout == "pipe"
    if not pipe_only:
        wvq = eperm(np.concatenate([Wv, Wq], axis=1))
        wvk = eperm(np.concatenate([Wv, Wk], axis=1))
        wkk = eperm(np.concatenate([Wk, Wk], axis=1))
        b0q = np.concatenate([z, bq])[:, None].copy()
        b0k = np.concatenate([z, bk])[:, None].copy()
    in_maps = []
    for c in range(NCORES):
        b, h = divmod(c, 2)
        own = x[b, h * HALF:(h + 1) * HALF].T        # [E, 1024]
        oth = x[b, (1 - h) * HALF:(2 - h) * HALF].T  # [E, 1024]
        # pipe layout: 8 x 256-token chunks (own x4 then oth x4), each
        # [128, ET, 256] with (p, et) holding e-row 8p+et.
        xt4 = np.ascontiguousarray(
            np.stack([own[:, i * 256:(i + 1) * 256] for i in range(4)]
                     + [oth[:, i * 256:(i + 1) * 256] for i in range(4)]
                     ).reshape(8, 128, ET, 256))
        m = {"xt4": xt4, "wkv2": wkv2, "wq": wqp, "bkv": bkv,
             "bq64": bq64, "bvb": bvb}
        if not pipe_only:
            # [2, 128, 4, 2048]: (half, p, chunk, j): e-row 8p+(chunk*2+j//1024)
            xt = np.stack([own, oth]).reshape(2, 128, 8, HALF)
            xt = np.ascontiguousarray(xt.reshape(2, 128, 4, ET * HALF // 4))
            m.update({"xt": xt, "wvq": wvq, "wk": wkk, "wvk": wvk,
                      "b0q": b0q, "b0k": b0k})
        in_maps.append(m)
    return in_maps


def gather_outputs(results):
    out = np.empty((B, S, H), np.float32)
    for c in range(NCORES):
        b, h = divmod(c, 2)
        oc = results[c]["out"]
        if oc.shape == (128, 8, H):  # pipe: token t*128+p at [p, t]
            oc = np.transpose(oc, (1, 0, 2)).reshape(HALF, H)
        out[b, h * HALF:(h + 1) * HALF] = oc
    return out


_NC_CACHE = {}


def _get_nc(mm_mode="f32r", reps=1, upto="full", layout="dup"):
    key = (mm_mode, reps, upto, layout)
    if key not in _NC_CACHE:
        _NC_CACHE[key] = build(mm_mode, reps, upto, layout)
    return _NC_CACHE[key]


def run(inputs, mm_mode="f32r", layout="cc", **kw):
    from concourse.bass_utils import run_bass_kernel_spmd

    nc = _get_nc(mm_mode, layout=layout)
    in_maps = shard_inputs(**inputs, mm_mode=mm_mode, layout=layout)
    res = run_bass_kernel_spmd(nc, in_maps, core_ids=list(range(NCORES)), **kw)
    return gather_outputs(res.results), res


def _build_exec(nc, in_maps):
    """Builds a re-invokable (non-donating) sharded executable + device args.

    Mirrors bass2jax.run_bass_via_pjrt's multi-core path, but keeps the
    output buffers as ordinary (non-donated) device arrays so the same
    callable can be executed repeatedly for wall-clock timing.
    """
    import jax
    from jax.sharding import Mesh, PartitionSpec, NamedSharding
    from jax.experimental.shard_map import shard_map
    from concourse import mybir
    from concourse.bass2jax import (
        _bass_exec_p, partition_id_tensor, install_neuronx_cc_hook,
    )

    install_neuronx_cc_hook()
    partition_name = nc.partition_id_tensor.name if nc.partition_id_tensor else None
    in_names, out_names, out_avals, zero_outs = [], [], [], []
    for alloc in nc.m.functions[0].allocations:
        if not isinstance(alloc, mybir.MemoryLocationSet):
            continue
        name = alloc.memorylocations[0].name
        if alloc.kind == "ExternalInput":
            if name != partition_name:
                in_names.append(name)
        elif alloc.kind == "ExternalOutput":
            out_names.append(name)
            shape = tuple(alloc.tensor_shape)
            dtype = mybir.dt.np(alloc.dtype)
            out_avals.append(jax.core.ShapedArray(shape, dtype))
            zero_outs.append(np.zeros(shape, dtype))
    n_params = len(in_names)
    all_in_names = list(in_names) + list(out_names)
    if partition_name is not None:
        all_in_names.append(partition_name)

    def _body(*args):
        operands = list(args)
        if partition_name is not None:
            operands.append(partition_id_tensor())
        outs = _bass_exec_p.bind(
            *operands,
            out_avals=tuple(out_avals),
            in_names=tuple(all_in_names),
            out_names=tuple(out_names),
            lowering_input_output_aliases=(),
            sim_require_finite=True,
            sim_require_nnan=True,
            nc=nc,
        )
        return tuple(outs)

    n_cores = len(in_maps)
    devices = jax.devices()[:n_cores]
    mesh = Mesh(np.asarray(devices), ("core",))
    nin = n_params + len(out_names)
    sharded = jax.jit(
        shard_map(
            _body, mesh=mesh,
            in_specs=(PartitionSpec("core"),) * nin,
            out_specs=(PartitionSpec("core"),) * len(out_names),
            check_rep=False,
        ),
        keep_unused=True,
    )
    sh = NamedSharding(mesh, PartitionSpec("core"))
    dev_args = [
        jax.device_put(
            np.concatenate([np.asarray(m[i]) for m in in_maps], axis=0), sh
        )
        for i in in_names
    ] + [
        jax.device_put(
            np.zeros((n_cores * z.shape[0], *z.shape[1:]), z.dtype), sh
        )
        for z in zero_outs
    ]
    return sharded, dev_args, out_names, out_avals


def _exec_results(r, out_names, out_avals):
    out_arrs = [np.asarray(a) for a in r]
    return [
        {
            name: out_arrs[i].reshape(NCORES, *out_avals[i].shape)[c]
            for i, name in enumerate(out_names)
        }
        for c in range(NCORES)
    ]


def bench(inputs, mm_mode="f32r", iters=50, reps=1, upto="full", layout="dup",
          n_cores=NCORES):
    """Amortized wall-clock per-execution time over repeated runs."""
    import jax, time

    nc = _get_nc(mm_mode, reps, upto, layout)
    in_maps = shard_inputs(**inputs, mm_mode=mm_mode)[:n_cores]
    fn, dev_args, out_names, out_avals = _build_exec(nc, in_maps)
    r = fn(*dev_args)
    jax.block_until_ready(r)  # compile + warm
    t0 = time.perf_counter()
    for _ in range(iters):
        r = fn(*dev_args)
    jax.block_until_ready(r)
    dt = (time.perf_counter() - t0) / iters
    if n_cores != NCORES:
        return None, dt
    return gather_outputs(_exec_results(r, out_names, out_avals)), dt


def kernel(**inputs) -> np.ndarray:
    try:
        out, _ = run(inputs, mm_mode="bf16", layout="pipe")
    except Exception:
        # Fall back to the proven collective-free data-parallel layout.
        out, _ = run(inputs, mm_mode="f32r", layout="dup")
    return out

